# revision 42
# baseline (speedup 1.0000x reference)
"""Trainium2 Bass kernel for nn_AlignmentMatrix.

Math: out[b,i,j] = ctx[b,i,:]@w1 + asp[b,j,:]@w2 + (ctx[b,i,:]*w3)@asp[b,j,:]
where w_u = cat([w1,w2,w3]).

Host-side refactor: fold everything into one batched matmul
    out[b].T = M_aug[b].T @ ctxT_aug[b]
with
    M_aug[b]    = [w3[:,None]*asp[b].T + w1[:,None] ; asp_term[b][None,:]]  (D+1, L2)
    ctxT_aug[b] = [ctx[b].T ; ones(1, L1)]                                   (D+1, L1)
    asp_term[b] = asp[b] @ w2
The device kernel is a pure streaming batched matmul in bf16 (f32 PSUM
accumulate), data-parallel over batch across 8 NeuronCores.  The host
pre-transposes ctx so the contraction dim lands on SBUF partitions, and
packs M + ctx into one partition-major stream so every DMA descriptor is
a single large contiguous read (descriptor-supply is the DMA bottleneck).
The device writes out^T in bf16; the host transposes/casts back.
"""

import numpy as np
import ml_dtypes

# Problem shape (hardcoded per spec)
B, L1, L2, D = 64, 512, 32, 600
NCORES = 8
NB = B // NCORES          # batches per core
KP = 128                  # partition chunk of contraction dim
NCH = 5                   # chunks
DP = KP * NCH             # 640 = padded D+1 (pad rows zero in M => no-op)
GRP = 4                   # batches per DMA group
MLEN = NCH * NB * L2      # 1280: m block elems per partition
XLEN = NCH * L1           # 2560: ctx elems per partition per batch
FREE = MLEN + NB * XLEN   # 21760 total free elems per partition
GLEN = GRP * XLEN         # 10240 per group

_CACHE = {}


def _ensure_profile_hook():
    """Register the NTFF profile hook so run(trace=True) works under axon."""
    import sys, types
    if 'antenv.axon_hooks' in sys.modules:
        return
    try:
        from trn_agent_boot.trn_boot import _ntff_profile_via_ctypes
        hook = _ntff_profile_via_ctypes('/opt/axon/libaxon_pjrt.so')
        mod = types.ModuleType('antenv.axon_hooks')
        mod.get_axon_ntff_profile_hook = lambda: hook
        sys.modules['antenv.axon_hooks'] = mod
    except Exception:
        pass


def _build_nc():
    """Build the per-core Bass graph (identical SPMD program for all 8 cores)."""
    import contextlib
    import concourse.bass as bass
    import concourse.mybir as mybir

    bf16 = mybir.dt.bfloat16
    f32 = mybir.dt.float32

    # Note: Bass.__init__'s const memsets + entry barrier cost ~3.5us but
    # act as a protective grace period for runtime init — removing or
    # shortening them produces NaN results or device hangs. Keep them.
    nc = bass.Bass()

    big_ext = nc.declare_dram_parameter("big", [KP, FREE], bf16, isOutput=False)
    # Device out layout: [p = (b%2)*32 + j, (b//2)*512 + i]; host decodes.
    out_ext = nc.declare_dram_parameter("out", [2 * L2, 4 * L1], bf16, isOutput=True)

    def moff(c, b):
        return (c * NB + b) * L2

    def xoff(b, c):
        return MLEN + b * XLEN + c * L1

    with contextlib.ExitStack() as ctx:
        NPAIR = NB // 2
        big_sb = ctx.enter_context(nc.sbuf_tensor("big_sb", [KP, FREE], bf16))
        # pairs 0-2 accumulate into one wide out tile, pair 3 in its own
        outA_sb = ctx.enter_context(nc.sbuf_tensor("outA_sb", [2 * L2, 3 * L1], bf16))
        outB_sb = ctx.enter_context(nc.sbuf_tensor("outB_sb", [2 * L2, L1], bf16))
        psums = [
            ctx.enter_context(nc.psum_tensor(f"ps{i}", [2 * L2, L1], f32))
            for i in range(NPAIR)
        ]
        ps_dummy = ctx.enter_context(nc.psum_tensor("ps_dummy", [L2, L1], f32))
        in_sem = ctx.enter_context(nc.semaphore("in_sem"))
        mm_sem = ctx.enter_context(nc.semaphore("mm_sem"))
        cp_sem = ctx.enter_context(nc.semaphore("cp_sem"))
        odma = ctx.enter_context(nc.semaphore("odma"))
        block = ctx.enter_context(nc.Block(no_gpsimd_drain=True))

        # Input DMA groups: (m+b0,b1), (b2,b3), (b4,b5), (b6,b7).
        # Each SDMA engine drains its descriptor share serially (~26GB/s),
        # with engine starts staggered ~4us by descriptor-write order, so
        # group sizes trade early first-batch against late last-batch.
        # Never split DMAs by partition range: a <128-partition DMA runs at
        # ~half the per-engine rate. Split along the free dim, FIFO one ring.
        cuts = [0] + [MLEN + k * XLEN for k in (2, 4, 6, 8)]
        NDMA = len(cuts) - 1
        # semaphore threshold required before starting pair q
        qwait = {0: 16, 1: 32, 2: 48, 3: 64}

        @block.sync
        def _(sync):
            for g in range(NDMA):
                sync.dma_start(
                    big_sb[:, cuts[g]:cuts[g + 1]], big_ext[:, cuts[g]:cuts[g + 1]]
                ).then_inc(in_sem, 16)
            # outA rides the (now idle) sync ring so its descriptor write
            # overlaps outB's on the scalar ring.
            sync.wait_ge(in_sem, 16 * NDMA)
            sync.wait_ge(cp_sem, 3)
            sync.dma_start(out_ext[:, :3 * L1], outA_sb[:]).then_inc(odma, 16)

        def warm(tensor, n):
            # Dummy matmuls into a dedicated PSUM bank keep the PE HAM clock
            # gate warm while waiting on input DMA groups.
            for _ in range(n):
                tensor.matmul(
                    ps_dummy[:],
                    big_sb[:, :L2],
                    big_sb[:, MLEN:MLEN + L1],
                    start=True,
                    stop=True,
                )

        @block.tensor
        def _(tensor):
            warm(tensor, 17)
            # Pairs of batches run concurrently on PE column groups 0 and 32,
            # accumulating into the two halves of one PSUM bank.
            for q in range(NPAIR):
                if q > 0:
                    warm(tensor, 6)
                tensor.wait_ge(in_sem, qwait[q])
                for c in range(NCH):
                    for h in range(2):
                        b = 2 * q + h
                        mm = tensor.matmul(
                            psums[q][h * L2:(h + 1) * L2, :],
                            big_sb[:, moff(c, b):moff(c, b) + L2],
                            big_sb[:, xoff(b, c):xoff(b, c) + L1],
                            start=(c == 0),
                            stop=(c == NCH - 1),
                            tile_position=(0, h * L2),
                        )
                        if c == NCH - 1 and h == 1:
                            mm.then_inc(mm_sem, 1)

        @block.vector
        def _(vector):
            for q in range(NPAIR):
                vector.wait_ge(mm_sem, q + 1)
                if q < 3:
                    dst = outA_sb[:, q * L1:(q + 1) * L1]
                else:
                    dst = outB_sb[:]
                vector.tensor_copy(dst, psums[q][:]).then_inc(cp_sem, 1)

        @block.scalar
        def _(scalar):
            # Gate outputs on input-stream completion: out descriptors
            # interleaving with input descriptors slows the stream ~20%.
            scalar.wait_ge(in_sem, 16 * NDMA)
            scalar.wait_ge(cp_sem, 4)
            scalar.dma_start(out_ext[:, 3 * L1:], outB_sb[:]).then_inc(odma, 16)
            scalar.wait_ge(odma, 32)

    nc.finalize()
    return nc


def _get_nc():
    if 'nc' not in _CACHE:
        _CACHE['nc'] = _build_nc()
    return _CACHE['nc']


def _prepare_in_maps(ctx, asp, w_u):
    ctx = np.asarray(ctx, dtype=np.float32)
    asp = np.asarray(asp, dtype=np.float32)
    w = np.asarray(w_u, dtype=np.float32).reshape(-1)
    w1, w2, w3 = w[:D], w[D:2 * D], w[2 * D:]

    # ctxT_aug padded to DP rows: [B, DP, L1]
    ctxt = np.empty((B, DP, L1), dtype=ml_dtypes.bfloat16)
    ctxt[:, :D, :] = ctx.transpose(0, 2, 1).astype(ml_dtypes.bfloat16)
    ctxt[:, D, :] = np.float32(1.0)
    ctxt[:, D + 1:, :] = 0
    # row (c*KP + p) -> [B, KP, NCH, L1] partition-major
    ctxt_pm = ctxt.reshape(B, NCH, KP, L1).transpose(0, 2, 1, 3)

    # M_aug padded: [B, DP, L2]
    m = np.zeros((B, DP, L2), dtype=np.float32)
    m[:, :D, :] = asp.transpose(0, 2, 1) * w3[None, :, None] + w1[None, :, None]
    m[:, D, :] = asp @ w2
    # [B, NCH, KP, L2]
    m_ck = m.astype(ml_dtypes.bfloat16).reshape(B, NCH, KP, L2)

    in_maps = []
    for core in range(NCORES):
        sl = slice(core * NB, (core + 1) * NB)
        # m block: [KP, NCH, NB, L2] -> [KP, MLEN]
        m_core = m_ck[sl].transpose(2, 1, 0, 3).reshape(KP, MLEN)
        # ctx block: [NB, KP, NCH, L1] -> [KP, NB, NCH, L1] -> [KP, NB*XLEN]
        x_core = ctxt_pm[sl].transpose(1, 0, 2, 3).reshape(KP, NB * XLEN)
        big = np.concatenate([m_core, x_core], axis=1)
        in_maps.append({"big": np.ascontiguousarray(big)})
    return in_maps


def run(inputs, trace=False, trace_kwargs=None):
    """Run the kernel on the full inputs; returns (out, BassKernelResults)."""
    from concourse import bass_utils
    from concourse.bass_utils import run_bass_kernel_spmd

    if trace:
        _ensure_profile_hook()
        bass_utils.upload_artifacts = lambda tmpdir: tmpdir

    in_maps = _prepare_in_maps(inputs["ctx"], inputs["asp"], inputs["w_u"])
    nc = _get_nc()
    res = run_bass_kernel_spmd(
        nc, in_maps, core_ids=list(range(NCORES)), trace=trace,
        **(trace_kwargs or {}),
    )
    # Gather: device out layout [p=(b%2)*32+j, (b//2)*512+i] in bf16.
    # Decode to outT[b, j, i], transpose to [b, i, j], concat cores.
    outs = []
    for i in range(NCORES):
        arr = np.asarray(res.results[i]["out"]).astype(np.float32)
        arr = arr.reshape(2, L2, 4, L1)          # [h, j, q, i]
        outT = arr.transpose(2, 0, 1, 3).reshape(NB, L2, L1)  # b = 2q + h
        outs.append(outT.transpose(0, 2, 1))
    return np.concatenate(outs, axis=0), res


def kernel(batch_size, ctx, asp, w_u):
    inputs = {"ctx": ctx, "asp": asp, "w_u": w_u}
    out, _ = run(inputs)
    if not np.isfinite(out).all():
        # Rare transient device glitch: retry once.
        out, _ = run(inputs)
    return out
